# revision 3
# baseline (speedup 1.0000x reference)
"""Trainium2 Bass kernel for nn_CustomFullyConnectedLayerGoogleTopK.

Reference computation:
    a = clip(K * softmax(alpha), 0, 1)                    # (4096,)
    W[rows, cols] += (V * a[:, None])  with rows=(j+i)%N, cols=j
    out = x @ W.T                                          # (256, 4096)

The scatter indices form a bijection (for each col j, row (j+i)%N hits every
row exactly once as i varies), so there is no actual accumulation:

    W[r, c] = V[(r - c) % N, c] * a[(r - c) % N]
    out[b, r] = sum_c x[b, c] * V[(r-c)%N, c] * a[(r-c)%N]

Sharding: output columns r are sharded 8 ways (512 per core) -> no collective;
each core reads only the diagonal band of V it needs, all of x, and produces a
disjoint out[:, r0:r0+512] slice.

The GEMM datapath runs in bf16 (tolerance is 2e-2; measured error ~4e-3).
Device-side layout trick: with the contraction rows presented in REVERSED
order (c = N-1-p for SBUF partition-row p), the skewed scale field the band
tiles need becomes the ascending Toeplitz  scale[p, j] = a2[1 + p + j]  where
a2 is `a` doubled.  Raw (doubled, rolled) alpha is DMA'd directly in this
overlapping-window layout; the soft-topk transform is applied on-chip:

    a = min(exp(alpha) * K/sum(exp(alpha)), 1)

Performance structure (learned from traces; the scored window is [first const
memset .. last teardown instruction], so pipeline-fill latency, the DMA ramp,
and the fixed ~6.5us semaphore-teardown postamble all count):

  * band and x^T ship pre-interleaved per contraction block in ONE dram
    tensor ("bxt", [128, NCB, 768] = 512 band cols + 256 xT cols per block).
  * The bulk stream is split across TWO HWDGE rings (sync + gpsimd), even
    batches on one and odd on the other, to parallelize both the ~650ns
    DMA_DIRECT2D issue cost and the per-queue descriptor dispatch.
  * Batches taper at the head (2,3,3 blocks then 4s, 2,2 at the tail) so the
    first matmuls start as early as possible and the last psum finishes
    close behind the final data.
  * alpha windows load in 5 chunks aligned to batch boundaries (2,6,8,8,8
    blocks; overlapping-AP reads of the doubled alpha vector); one Exp per
    chunk on the otherwise-idle Scalar engine, one fused mult+min
    tensor_scalar per chunk on Vector (bf16 single-src -> 4x mode), with
    1/sum from a single-pass bf16 ones-matmul partition broadcast + DVE
    reciprocal.
  * Engine FIFO order is load-bearing everywhere: every queue gets its ops
    in expected-arrival order (a wait mid-queue blocks everything behind
    it).  Keep-alive matmuls gated on early DMA arrivals stop the HAM
    activity monitor from clock-throttling the PE between the warmup burst
    and the first real matmul batch (cold matmuls run 427-634ns vs 216ns).
  * Tail: psum1's matmuls for the last three batches are deferred until
    after psum0 completes, so psum0's cast+store (DVE cast + gpsimd-ring
    store) fully overlaps them; psum1 then takes the ACT-copy + sync-ring
    store path.
"""

import os
import sys

import numpy as np

for _p in ("/opt/trn_rl_repo", "/root/.axon_site/_ro/trn_rl_repo"):
    if os.path.isdir(_p) and _p not in sys.path:
        sys.path.append(_p)

import ml_dtypes

import concourse.bacc as bacc
import concourse.bass as bass
import concourse.mybir as mybir
import concourse.tile as tile
from concourse.bass_utils import run_bass_kernel_spmd

F32 = mybir.dt.float32
BF16 = mybir.dt.bfloat16
NP_BF16 = ml_dtypes.bfloat16

N = 4096          # IN_F == OUT_F == N_PERM == DIAG
B = 256           # batch
NCORES = 8
RW = N // NCORES  # 512 output columns per core
K_TOPK = 3687     # ceil(0.9 * 4096 * 4096 / 4096)
CB = 128          # contraction block (SBUF partition count)
NCB = N // CB     # 32 contraction blocks
BW = 512 + 256    # interleaved row: band cols + xT cols per block

# window segments (start block, n blocks) and bxt batches (start, n, seg idx)
SEGS = [(0, 2), (2, 6), (8, 8), (16, 8), (24, 8)]
BATCHES = [
    (0, 2, 0), (2, 3, 1), (5, 3, 1),
    (8, 4, 2), (12, 4, 2), (16, 4, 3), (20, 4, 3),
    (24, 4, 4), (28, 2, 4), (30, 2, 4),
]
NBQ = len(BATCHES)
SEG_FIRST_BATCH = [0, 1, 3, 5, 7]   # batch index that first needs each seg
TAIL_Q0 = 7                          # batches >= this defer psum1 matmuls


def _strided_cols(ap2d, col_off, t_step, n_t, inner):
    """[128, W] SBUF tile -> [128, n_t, inner] view starting at col_off with
    column stride t_step between t-slices (overlap allowed)."""
    pstep = ap2d.ap[0][0]
    return bass.AP(
        ap2d.tensor, ap2d.offset + col_off,
        [[pstep, 128], [t_step, n_t], [1, inner]],
    )


def _build_program():
    nc = bacc.Bacc("TRN2", target_bir_lowering=False, debug=False)

    bxt = nc.dram_tensor("bxt", [128, NCB, BW], BF16, kind="ExternalInput").ap()
    alpha2 = nc.dram_tensor("alpha2", [2 * N], BF16, kind="ExternalInput").ap()
    out = nc.dram_tensor("out", [B, RW], BF16, kind="ExternalOutput").ap()

    with tile.TileContext(nc) as tc:
        with (
            tc.tile_pool(name="small", bufs=1) as sp,
            tc.tile_pool(name="graw", bufs=1) as grp,
            tc.tile_pool(name="gexp", bufs=1) as gxp,
            tc.tile_pool(name="gsc", bufs=1) as gwp,
            tc.tile_pool(name="bxtp", bufs=1) as bxp,
            tc.tile_pool(name="wt", bufs=4) as wtp,
            tc.tile_pool(name="opool", bufs=2) as op,
            tc.tile_pool(name="psum", bufs=1, space="PSUM") as pp,
            tc.tile_pool(name="psum_s", bufs=1, space="PSUM") as pps,
        ):
            # ---- input DMAs: TWO rings, strict need-order on each ----
            # ring A (sync):   alpha, w0, b0, w2, b2, b4, w4, b6, b8
            # ring B (gpsimd): w1, b1, w3, b3, b5, b7, b9
            alpha_sb = sp.tile([128, 2 * N // 128], BF16)
            nc.sync.dma_start(
                alpha_sb[:], alpha2[0 : 2 * N].rearrange("(p f) -> p f", p=128)
            )
            graw = [
                grp.tile([128, RW + (sz - 1) * CB], BF16, name=f"graw{s}")
                for s, (_, sz) in enumerate(SEGS)
            ]
            bxt_sb = bxp.tile([128, NCB, BW], BF16)

            def _dma_win(eng, s):
                k0, sz = SEGS[s]
                src = bass.AP(
                    alpha2.tensor,
                    alpha2.offset + 1 + k0 * CB,
                    [[1, 128], [1, RW + (sz - 1) * CB]],
                )
                eng.dma_start(graw[s][:], src)

            def _dma_bxt(eng, q):
                k0, nb, _ = BATCHES[q]
                eng.dma_start(
                    bxt_sb[:, k0 : k0 + nb, :], bxt[:, k0 : k0 + nb, :]
                )

            A, Bq = nc.sync, nc.gpsimd
            _dma_win(A, 0)
            _dma_win(Bq, 1)
            _dma_bxt(A, 0)
            _dma_bxt(Bq, 1)
            _dma_win(A, 2)
            _dma_win(Bq, 3)
            _dma_bxt(A, 2)
            _dma_bxt(Bq, 3)
            _dma_bxt(A, 4)
            _dma_bxt(Bq, 5)
            _dma_win(A, 4)
            _dma_bxt(A, 6)
            _dma_bxt(Bq, 7)
            _dma_bxt(A, 8)
            _dma_bxt(Bq, 9)

            # ---- PE warmup: HAM clock ramps before the first real matmul ----
            ones = sp.tile([128, 128], BF16)
            nc.vector.memset(ones[:], 1.0)
            psum_ka = pps.tile([128, 1], F32)
            for _ in range(6):
                nc.tensor.matmul(
                    psum_ka[:], ones[:], ones[:, 0:1], start=True, stop=True
                )
            # keep-alive gated on the first window's arrival (~9.4us)
            nc.tensor.matmul(
                psum_ka[:], graw[0][:, 0:128], graw[0][:, 0:1],
                start=True, stop=True,
            )

            # ---- kinv = K / sum(exp(alpha)) broadcast to all partitions ----
            exp_sb = sp.tile([128, 2 * N // 128], BF16)
            rowsum = sp.tile([128, 1], F32)
            # alpha is uniform in [0,1): no max-subtraction needed
            nc.scalar.activation(
                exp_sb[:], alpha_sb[:], mybir.ActivationFunctionType.Exp,
                accum_out=rowsum[:],
            )
            rowsum_bf = sp.tile([128, 1], BF16)
            nc.vector.tensor_copy(rowsum_bf[:], rowsum[:])
            tot_ps = pps.tile([128, 1], F32)
            # total = ones.T @ rowsum -> per-partition copy of 2*sum (bf16
            # operands -> single-pass matmul; error ~0.4%/sqrt(128), negligible)
            nc.tensor.matmul(
                tot_ps[:], ones[:], rowsum_bf[:], start=True, stop=True
            )
            inv = sp.tile([128, 1], F32)
            nc.vector.reciprocal(inv[:], tot_ps[:])
            kinv = sp.tile([128, 1], F32)
            # rowsum covered the doubled alpha -> tot = 2*sum, so scale by 2K
            nc.vector.tensor_scalar_mul(kinv[:], inv[:], 2.0 * K_TOPK)

            # window exps ride the Scalar queue upfront in arrival order;
            # Scalar has no other mid-kernel work
            agx = [
                gxp.tile([128, RW + (sz - 1) * CB], BF16, name=f"agx{s}")
                for s, (_, sz) in enumerate(SEGS)
            ]
            for s in range(len(SEGS)):
                nc.scalar.activation(
                    agx[s][:], graw[s][:], mybir.ActivationFunctionType.Exp
                )

            # ---- main loop ----
            agw = [
                gwp.tile([128, RW + (sz - 1) * CB], BF16, name=f"agw{s}")
                for s, (_, sz) in enumerate(SEGS)
            ]
            psum0 = pp.tile([128, RW], F32)
            psum1 = pp.tile([128, RW], F32)
            wts = []
            for q, (k0, nb, s) in enumerate(BATCHES):
                if q == SEG_FIRST_BATCH[s]:
                    # scale chunk s: fused *kinv, min-1 on Vector (bf16
                    # single-src -> 4x mode); emitted just-in-time so the
                    # Vector FIFO never blocks a ready TT behind it
                    nc.vector.tensor_scalar(
                        agw[s][:], agx[s][:], kinv[:, 0:1], 1.0,
                        mybir.AluOpType.mult, mybir.AluOpType.min,
                    )
                if q > 0:
                    # PE keep-alive gated on this batch's arrival
                    nc.tensor.matmul(
                        psum_ka[:], bxt_sb[:, k0, 0:128], bxt_sb[:, k0, 0:1],
                        start=True, stop=True,
                    )
                # scaled weights for this batch of nb contraction blocks
                wt = wtp.tile([128, 4, RW], BF16)
                wts.append(wt)
                nc.vector.tensor_tensor(
                    wt[:, 0:nb, :],
                    bxt_sb[:, k0 : k0 + nb, 0:RW],
                    _strided_cols(agw[s], (k0 - SEGS[s][0]) * CB, CB, nb, RW),
                    mybir.AluOpType.mult,
                )
                for i in range(nb):
                    t = k0 + i
                    nc.tensor.matmul(
                        psum0[:], bxt_sb[:, t, 512:640], wt[:, i, :],
                        start=(t == 0), stop=(t == NCB - 1),
                    )
                    if q < TAIL_Q0:
                        nc.tensor.matmul(
                            psum1[:], bxt_sb[:, t, 640:768], wt[:, i, :],
                            start=(t == 0), stop=False,
                        )

            # psum0 done: cast + store on the DVE/gpsimd-ring path while
            # psum1's deferred tail matmuls run
            o0 = op.tile([128, RW], BF16)
            nc.vector.tensor_copy(o0[:], psum0[:])
            nc.gpsimd.dma_start(out[0:128, :], o0[:])
            for q in range(TAIL_Q0, NBQ):
                k0, nb, _ = BATCHES[q]
                for i in range(nb):
                    t = k0 + i
                    nc.tensor.matmul(
                        psum1[:], bxt_sb[:, t, 640:768], wts[q][:, i, :],
                        start=False, stop=(t == NCB - 1),
                    )
            o1 = op.tile([128, RW], BF16)
            nc.scalar.activation(
                o1[:], psum1[:], mybir.ActivationFunctionType.Copy
            )
            nc.sync.dma_start(out[128:256, :], o1[:])

    nc.compile()
    return nc


_NC_CACHE = []


def _get_program():
    if not _NC_CACHE:
        _NC_CACHE.append(_build_program())
    return _NC_CACHE[0]


def prepare_in_maps(x: np.ndarray, V: np.ndarray, alpha: np.ndarray):
    """Layout/dtype-only sharding of the full inputs into 8 per-core maps."""
    x = np.ascontiguousarray(np.asarray(x, dtype=np.float32))
    V = np.ascontiguousarray(np.asarray(V, dtype=np.float32))
    alpha = np.ascontiguousarray(np.asarray(alpha, dtype=np.float32))

    # rows presented in reversed order (c = N-1-p); see module docstring.
    # blocked [128, NCB, B] so each DMA chunk is contiguous per partition.
    xTb = np.ascontiguousarray(
        x.T[::-1, :].reshape(NCB, 128, B).transpose(1, 0, 2)
    ).astype(NP_BF16)

    # VtD[c, t] = V[t % N, c] for t in [0, 2N): doubled transpose for wrap-free
    # band extraction. band_m[c, j] = V[(r0 + j - c) % N, c]
    #              = VtD[c, N + r0 + j - c]
    Vt = np.ascontiguousarray(V.T)
    VtD = np.concatenate([Vt, Vt], axis=1)  # (N, 2N)
    flat = VtD.reshape(-1)
    isz = flat.itemsize

    in_maps = []
    for m in range(NCORES):
        r0 = m * RW
        start = N + r0  # element offset of band_m[0, 0] in flat
        band_m = np.lib.stride_tricks.as_strided(
            flat[start:], shape=(N, RW), strides=((2 * N - 1) * isz, isz),
        )
        band_b = np.ascontiguousarray(
            band_m[::-1, :].reshape(NCB, 128, RW).transpose(1, 0, 2)
        ).astype(NP_BF16)
        bxt_b = np.concatenate([band_b, xTb], axis=2)  # [128, NCB, 768]
        am = np.roll(alpha, -r0)
        in_maps.append({
            "bxt": np.ascontiguousarray(bxt_b),
            "alpha2": np.concatenate([am, am]).astype(NP_BF16),
        })
    return in_maps


def gather_output(results) -> np.ndarray:
    return np.concatenate(
        [np.asarray(results[m]["out"], dtype=np.float32) for m in range(NCORES)],
        axis=1,
    )


def kernel(x: np.ndarray, V: np.ndarray, alpha: np.ndarray) -> np.ndarray:
    in_maps = prepare_in_maps(x, V, alpha)
    nc = _get_program()
    res = run_bass_kernel_spmd(nc, in_maps, core_ids=list(range(NCORES)))
    return gather_output(res.results)


# revision 7
# speedup vs baseline: 1.1987x; 1.1987x over previous
"""Trainium2 Bass kernel for nn_CustomFullyConnectedLayerGoogleTopK.

Reference computation:
    a = clip(K * softmax(alpha), 0, 1)                    # (4096,)
    W[rows, cols] += (V * a[:, None])  with rows=(j+i)%N, cols=j
    out = x @ W.T                                          # (256, 4096)

The scatter indices form a bijection (for each col j, row (j+i)%N hits every
row exactly once as i varies), so there is no actual accumulation:

    W[r, c] = V[(r - c) % N, c] * a[(r - c) % N]
    out[b, r] = sum_c x[b, c] * V[(r-c)%N, c] * a[(r-c)%N]

Sharding: output columns r are sharded 8 ways (512 per core) -> no collective;
each core reads only the diagonal band of V it needs, all of x, and produces a
disjoint out[:, r0:r0+512] slice.

The GEMM datapath runs in bf16 (tolerance is 2e-2; measured error ~4e-3).
Device-side layout trick: with the contraction rows presented in REVERSED
order (c = N-1-p for SBUF partition-row p), the skewed scale field the band
tiles need becomes the ascending Toeplitz  scale[p, j] = a2[1 + p + j]  where
a2 is `a` doubled.  Raw (doubled, rolled) alpha is DMA'd directly in this
overlapping-window layout; the soft-topk transform is applied on-chip:

    a = min(exp(alpha) * K/sum(exp(alpha)), 1)

Performance structure (learned from traces; the scored window is [first const
memset .. last teardown instruction], so pipeline-fill latency, the DMA ramp,
and the fixed ~6.5us semaphore-teardown postamble all count):

  * band and x^T ship pre-interleaved per contraction block in ONE dram
    tensor ("bxt", [128, NCB, 768] = 512 band cols + 256 xT cols per block).
  * ALL input loads ride ONE HWDGE ring (sync) in strict need order:
    single-queue FIFO makes completion order == need order.  (Measured: a
    second bulk ring DROPS aggregate throughput ~20% — the ~430GB/s cap is
    shared — and per-descriptor round-robin starves whichever ring has the
    smaller descriptors by ~4x.)
  * Batches taper at the head (2,3,3 blocks then 4s, 2,2 at the tail) so the
    first matmuls start as early as possible and the last psum finishes
    close behind the final data.
  * alpha windows load in 5 chunks aligned to batch boundaries (2,6,8,8,8
    blocks; overlapping-AP reads of the doubled alpha vector); one Exp per
    chunk on the otherwise-idle Scalar engine, one fused mult+min
    tensor_scalar per chunk on Vector (bf16 single-src -> 4x mode), with
    1/sum from a single-pass bf16 ones-matmul partition broadcast + DVE
    reciprocal.
  * Engine FIFO order is load-bearing everywhere: every queue gets its ops
    in expected-arrival order (a wait mid-queue blocks everything behind
    it).  Keep-alive matmuls gated on early DMA arrivals stop the HAM
    activity monitor from clock-throttling the PE between the warmup burst
    and the first real matmul batch (cold matmuls run 427-634ns vs 216ns).
  * Tail: psum1's matmuls for the last three batches are deferred until
    after psum0 completes, so psum0's cast+store (DVE cast + gpsimd-ring
    store) fully overlaps them; psum1 then takes the ACT-copy + sync-ring
    store path.
"""

import os
import sys

import numpy as np

for _p in ("/opt/trn_rl_repo", "/root/.axon_site/_ro/trn_rl_repo"):
    if os.path.isdir(_p) and _p not in sys.path:
        sys.path.append(_p)

import ml_dtypes

import concourse.bacc as bacc
import concourse.bass as bass
import concourse.mybir as mybir
import concourse.tile as tile
from concourse.bass_utils import run_bass_kernel_spmd

F32 = mybir.dt.float32
BF16 = mybir.dt.bfloat16
NP_BF16 = ml_dtypes.bfloat16

N = 4096          # IN_F == OUT_F == N_PERM == DIAG
B = 256           # batch
NCORES = 8
RW = N // NCORES  # 512 output columns per core
K_TOPK = 3687     # ceil(0.9 * 4096 * 4096 / 4096)
CB = 128          # contraction block (SBUF partition count)
NCB = N // CB     # 32 contraction blocks
BW = 512 + 256    # interleaved row: band cols + xT cols per block

# window segments (start block, n blocks) and bxt batches (start, n, seg idx)
SEGS = [(0, 2), (2, 6), (8, 8), (16, 8), (24, 8)]
BATCHES = [
    (0, 2, 0), (2, 3, 1), (5, 3, 1),
    (8, 4, 2), (12, 4, 2), (16, 4, 3), (20, 4, 3),
    (24, 4, 4), (28, 2, 4), (30, 2, 4),
]
NBQ = len(BATCHES)
SEG_FIRST_BATCH = [0, 1, 3, 5, 7]   # batch index that first needs each seg
TAIL_Q0 = 7                          # batches >= this defer psum1 matmuls


def _strided_cols(ap2d, col_off, t_step, n_t, inner):
    """[128, W] SBUF tile -> [128, n_t, inner] view starting at col_off with
    column stride t_step between t-slices (overlap allowed)."""
    pstep = ap2d.ap[0][0]
    return bass.AP(
        ap2d.tensor, ap2d.offset + col_off,
        [[pstep, 128], [t_step, n_t], [1, inner]],
    )


def _build_program():
    nc = bacc.Bacc("TRN2", target_bir_lowering=False, debug=False)

    bxt = nc.dram_tensor("bxt", [128, NCB, BW], BF16, kind="ExternalInput").ap()
    alpha2 = nc.dram_tensor("alpha2", [2 * N], BF16, kind="ExternalInput").ap()
    out = nc.dram_tensor("out", [B, RW], BF16, kind="ExternalOutput").ap()

    with tile.TileContext(nc) as tc:
        with (
            tc.tile_pool(name="small", bufs=1) as sp,
            tc.tile_pool(name="graw", bufs=1) as grp,
            tc.tile_pool(name="gexp", bufs=1) as gxp,
            tc.tile_pool(name="gsc", bufs=1) as gwp,
            tc.tile_pool(name="bxtp", bufs=1) as bxp,
            tc.tile_pool(name="wt", bufs=4) as wtp,
            tc.tile_pool(name="opool", bufs=2) as op,
            tc.tile_pool(name="psum", bufs=1, space="PSUM") as pp,
            tc.tile_pool(name="psum_s", bufs=1, space="PSUM") as pps,
        ):
            # ---- input DMAs: ONE ring (sync), strict need order ----
            # [alpha, w0, b0, w1, b1, b2, w2, b3, b4, w3, b5, b6, w4, b7-b9]
            alpha_sb = sp.tile([128, 2 * N // 128], BF16)
            nc.sync.dma_start(
                alpha_sb[:], alpha2[0 : 2 * N].rearrange("(p f) -> p f", p=128)
            )
            graw = [
                grp.tile([128, RW + (sz - 1) * CB], BF16, name=f"graw{s}")
                for s, (_, sz) in enumerate(SEGS)
            ]
            bxt_sb = bxp.tile([128, NCB, BW], BF16)

            def _dma_win(eng, s):
                k0, sz = SEGS[s]
                src = bass.AP(
                    alpha2.tensor,
                    alpha2.offset + 1 + k0 * CB,
                    [[1, 128], [1, RW + (sz - 1) * CB]],
                )
                eng.dma_start(graw[s][:], src)

            def _dma_bxt(eng, q):
                k0, nb, _ = BATCHES[q]
                eng.dma_start(
                    bxt_sb[:, k0 : k0 + nb, :], bxt[:, k0 : k0 + nb, :]
                )

            A = nc.sync
            _dma_win(A, 0)
            _dma_bxt(A, 0)
            _dma_win(A, 1)
            _dma_bxt(A, 1)
            _dma_bxt(A, 2)
            _dma_win(A, 2)
            _dma_bxt(A, 3)
            _dma_bxt(A, 4)
            _dma_win(A, 3)
            _dma_bxt(A, 5)
            _dma_bxt(A, 6)
            _dma_win(A, 4)
            _dma_bxt(A, 7)
            _dma_bxt(A, 8)
            _dma_bxt(A, 9)

            # ---- PE warmup: HAM clock ramps before the first real matmul ----
            ones = sp.tile([128, 128], BF16)
            nc.vector.memset(ones[:], 1.0)
            psum_ka = pps.tile([128, 1], F32)
            for _ in range(6):
                nc.tensor.matmul(
                    psum_ka[:], ones[:], ones[:, 0:1], start=True, stop=True
                )

            # ---- kinv = K / sum(exp(alpha)) broadcast to all partitions ----
            exp_sb = sp.tile([128, 2 * N // 128], BF16)
            rowsum = sp.tile([128, 1], F32)
            # alpha is uniform in [0,1): no max-subtraction needed
            nc.scalar.activation(
                exp_sb[:], alpha_sb[:], mybir.ActivationFunctionType.Exp,
                accum_out=rowsum[:],
            )
            rowsum_bf = sp.tile([128, 1], BF16)
            nc.vector.tensor_copy(rowsum_bf[:], rowsum[:])
            # keep-alive gated on the first window's arrival, which lands
            # just before rowsum_bf is ready (Tensor queue is in-order:
            # every gate must fire before the next instruction's data would)
            nc.tensor.matmul(
                psum_ka[:], graw[0][:, 0:128], graw[0][:, 0:1],
                start=True, stop=True,
            )
            tot_ps = pps.tile([128, 1], F32)
            # total = ones.T @ rowsum -> per-partition copy of 2*sum (bf16
            # operands -> single-pass matmul; error ~0.4%/sqrt(128), negligible)
            nc.tensor.matmul(
                tot_ps[:], ones[:], rowsum_bf[:], start=True, stop=True
            )
            inv = sp.tile([128, 1], F32)
            nc.vector.reciprocal(inv[:], tot_ps[:])
            kinv = sp.tile([128, 1], F32)
            # rowsum covered the doubled alpha -> tot = 2*sum, so scale by 2K
            nc.vector.tensor_scalar_mul(kinv[:], inv[:], 2.0 * K_TOPK)
            # keep-alive gated on the first bxt batch's arrival
            nc.tensor.matmul(
                psum_ka[:], bxt_sb[:, 0, 0:128], bxt_sb[:, 0, 0:1],
                start=True, stop=True,
            )

            # window exps ride the Scalar queue upfront in arrival order;
            # Scalar has no other mid-kernel work
            agx = [
                gxp.tile([128, RW + (sz - 1) * CB], BF16, name=f"agx{s}")
                for s, (_, sz) in enumerate(SEGS)
            ]
            for s in range(len(SEGS)):
                nc.scalar.activation(
                    agx[s][:], graw[s][:], mybir.ActivationFunctionType.Exp
                )

            # ---- main loop ----
            agw = [
                gwp.tile([128, RW + (sz - 1) * CB], BF16, name=f"agw{s}")
                for s, (_, sz) in enumerate(SEGS)
            ]
            psum0 = pp.tile([128, RW], F32)
            psum1 = pp.tile([128, RW], F32)
            wts = []
            for q, (k0, nb, s) in enumerate(BATCHES):
                if q == SEG_FIRST_BATCH[s]:
                    # scale chunk s: fused *kinv, min-1 on Vector (bf16
                    # single-src -> 4x mode); emitted just-in-time so the
                    # Vector FIFO never blocks a ready TT behind it
                    nc.vector.tensor_scalar(
                        agw[s][:], agx[s][:], kinv[:, 0:1], 1.0,
                        mybir.AluOpType.mult, mybir.AluOpType.min,
                    )
                if q > 0:
                    # PE keep-alive gated on this batch's arrival
                    nc.tensor.matmul(
                        psum_ka[:], bxt_sb[:, k0, 0:128], bxt_sb[:, k0, 0:1],
                        start=True, stop=True,
                    )
                # scaled weights for this batch of nb contraction blocks
                wt = wtp.tile([128, 4, RW], BF16)
                wts.append(wt)
                nc.vector.tensor_tensor(
                    wt[:, 0:nb, :],
                    bxt_sb[:, k0 : k0 + nb, 0:RW],
                    _strided_cols(agw[s], (k0 - SEGS[s][0]) * CB, CB, nb, RW),
                    mybir.AluOpType.mult,
                )
                for i in range(nb):
                    t = k0 + i
                    nc.tensor.matmul(
                        psum0[:], bxt_sb[:, t, 512:640], wt[:, i, :],
                        start=(t == 0), stop=(t == NCB - 1),
                    )
                    if q < TAIL_Q0:
                        nc.tensor.matmul(
                            psum1[:], bxt_sb[:, t, 640:768], wt[:, i, :],
                            start=(t == 0), stop=False,
                        )

            # psum0 done: cast + store on the DVE/gpsimd-ring path while
            # psum1's deferred tail matmuls run
            o0 = op.tile([128, RW], BF16)
            nc.vector.tensor_copy(o0[:], psum0[:])
            nc.gpsimd.dma_start(out[0:128, :], o0[:])
            for q in range(TAIL_Q0, NBQ):
                k0, nb, _ = BATCHES[q]
                for i in range(nb):
                    t = k0 + i
                    nc.tensor.matmul(
                        psum1[:], bxt_sb[:, t, 640:768], wts[q][:, i, :],
                        start=False, stop=(t == NCB - 1),
                    )
            o1 = op.tile([128, RW], BF16)
            nc.scalar.activation(
                o1[:], psum1[:], mybir.ActivationFunctionType.Copy
            )
            nc.sync.dma_start(out[128:256, :], o1[:])

    nc.compile()
    return nc


_NC_CACHE = []


def _get_program():
    if not _NC_CACHE:
        _NC_CACHE.append(_build_program())
    return _NC_CACHE[0]


def prepare_in_maps(x: np.ndarray, V: np.ndarray, alpha: np.ndarray):
    """Layout/dtype-only sharding of the full inputs into 8 per-core maps."""
    x = np.ascontiguousarray(np.asarray(x, dtype=np.float32))
    V = np.ascontiguousarray(np.asarray(V, dtype=np.float32))
    alpha = np.ascontiguousarray(np.asarray(alpha, dtype=np.float32))

    # rows presented in reversed order (c = N-1-p); see module docstring.
    # blocked [128, NCB, B] so each DMA chunk is contiguous per partition.
    xTb = np.ascontiguousarray(
        x.T[::-1, :].reshape(NCB, 128, B).transpose(1, 0, 2)
    ).astype(NP_BF16)

    # VtD[c, t] = V[t % N, c] for t in [0, 2N): doubled transpose for wrap-free
    # band extraction. band_m[c, j] = V[(r0 + j - c) % N, c]
    #              = VtD[c, N + r0 + j - c]
    Vt = np.ascontiguousarray(V.T)
    VtD = np.concatenate([Vt, Vt], axis=1)  # (N, 2N)
    flat = VtD.reshape(-1)
    isz = flat.itemsize

    in_maps = []
    for m in range(NCORES):
        r0 = m * RW
        start = N + r0  # element offset of band_m[0, 0] in flat
        band_m = np.lib.stride_tricks.as_strided(
            flat[start:], shape=(N, RW), strides=((2 * N - 1) * isz, isz),
        )
        band_b = np.ascontiguousarray(
            band_m[::-1, :].reshape(NCB, 128, RW).transpose(1, 0, 2)
        ).astype(NP_BF16)
        bxt_b = np.concatenate([band_b, xTb], axis=2)  # [128, NCB, 768]
        am = np.roll(alpha, -r0)
        in_maps.append({
            "bxt": np.ascontiguousarray(bxt_b),
            "alpha2": np.concatenate([am, am]).astype(NP_BF16),
        })
    return in_maps


def gather_output(results) -> np.ndarray:
    return np.concatenate(
        [np.asarray(results[m]["out"], dtype=np.float32) for m in range(NCORES)],
        axis=1,
    )


def kernel(x: np.ndarray, V: np.ndarray, alpha: np.ndarray) -> np.ndarray:
    in_maps = prepare_in_maps(x, V, alpha)
    nc = _get_program()
    res = run_bass_kernel_spmd(nc, in_maps, core_ids=list(range(NCORES)))
    return gather_output(res.results)
